# revision 26
# baseline (speedup 1.0000x reference)
"""Trainium2 Bass kernel for nn_EnhancedGatedTemporalFusion.

Mathematical structure exploited (all exact at f32 precision):
  * The self-attention block in the reference is dead code (its result is
    never used downstream), so it is skipped.
  * The output weighting is softmax(arange(S,0,-1)), i.e. w[t] = exp(-t)/Z.
    Since |outputs[t]| <= 2 (convex combinations of tanh values), the tail
    sum over t >= T is bounded by 2*e^{-T}; at T=32 that is ~2.5e-14
    absolute against a result of magnitude ~0.05 (f32 ulp ~4e-9), below one
    ulp; the fp16 storage of w[t] zeroes t>=17 anyway (tail ~2e-6 relative,
    far below the fp16 weight-quantization noise).
  * The gated update h' = g*h + (1-g)*c is an affine recurrence that maps
    1:1 onto the DVE TensorTensorScanArith instruction
    (state = (g mult state) add u, scanned along the free dimension).
  * When b_pe1 == 0 and positions >= 0 (true for this problem's inputs),
    relu(pos_t*w1[h]) = pos_t*relu(w1[h]), so the positional-encoding MLP
    is rank-1: pos_emb[t, :] = pos_t * (W_pe2 @ relu(W_pe1[:, 0])).  That
    O(T*IN_D) correction is folded into the x input on the host, removing
    the whole 1024-wide pe stage from the device graph.  A general device
    path is kept as a fallback and used automatically if the guard fails.

Sharding across the 8 cores: the hidden dim H=1024 is split 128 lanes per
core.  Each core computes the (replicated, tiny) 300-wide projection, its
own h-slice of the gate/candidate GEMMs, the scan, the exp-weighted time
reduction, and a partial product of the final H->2613 projection over its
h-slice.  The 8 partial vectors are summed on the host (contraction
unshard) and the output bias is added.

Layout notes: device inputs are pre-packed on the host into a few DMA
transfers ordered by when the consuming compute needs them (each dma_start
costs ~625ns of HWDGE issue time, so many small DMAs serialize the front).
The final projection keeps the output dim on PSUM partitions ((128, 21)
tile, d = 128*j + p) so the PSUM->SBUF copy is partition-parallel; the
host undoes that tiling when gathering.
"""

import sys

import numpy as np

if "/opt/trn_rl_repo" not in sys.path:
    sys.path.insert(0, "/opt/trn_rl_repo")

T = 32           # truncated horizon (exact under f32, see module docstring)
IN_D = 100       # input_dim
CH = 300         # proj1 out dim
H = 1024         # hidden dim
OUT_D = 2613     # output dim
OUT_PAD = 2688   # 21 * 128
NJ = OUT_PAD // 128
NCORES = 8
HSL = H // NCORES      # h-lanes per core
NT = H // 128          # h tiles of 128 for the fallback pe stage
NKC = CH // 100        # contraction chunks of 100 for the 300-dim

# pack128 free-dim offsets (wb + per-core gate biases)
_P128_WB = 0
_P128_BGC = _P128_WB + T
_P128_LEN = _P128_BGC + 4

# fast path: packA carries wb + gate biases + xT + fused gate weights.
# wb, xT and the gate weights are fp16 bit-packed into the f32 pack (halves
# the front DMA, runs gate matmuls at 1 cycle/row, 2x DVE modes downstream).
# pack1: fp16 [xT | ones-row] (101, T) + fp16 augmented gate weights
# (101, 4*HSL) with the fused biases in row 100 (K=101 matmul adds them).
_P1_XT = 0
_P1_WGF = _P1_XT + T // 2
_P1_WB = _P1_WGF + 4 * HSL // 2       # fp16 softmax weights, duplicated (128, 2T)
_P1_LEN = _P1_WB + T

# general-path offsets
_P100_XT = 0
_P100_WP1 = _P100_XT + T
_P100_BP1 = _P100_WP1 + CH
_P100A_LEN = _P100_BP1 + NKC
_WG_LEN = 4 * NKC * HSL
_P100_WG = _P100A_LEN
_P100_LEN = _P100_WG + _WG_LEN

_CACHE = {}


def _build_core(nc, tile, mybir, tc, cst, pmm, pout, xT, wp1, bp1, wgt, bgc, wb, wout, d_out):
    """Shared back end: combined -> gates -> scan -> weighted sum -> partial out."""
    f32 = mybir.dt.float32
    AF = mybir.ActivationFunctionType
    OP = mybir.AluOpType

    # combinedT (ch=300 in 3 chunks of 100, t)
    combT = cst.tile([100, NKC, T], f32)
    for ch in range(NKC):
        pcomb = pmm.tile([100, T], f32, tag="mm", name=f"pcomb{ch}")
        nc.tensor.matmul(
            pcomb, wp1[:, ch * 100 : (ch + 1) * 100], xT, start=True, stop=True
        )
        nc.vector.tensor_scalar_add(combT[:, ch, :], pcomb, bp1[:, ch : ch + 1])

    def gate(nm, func, bcol):
        pg = pmm.tile([HSL, T], f32, tag="mm", name=f"p_{nm}")
        for ch in range(NKC):
            nc.tensor.matmul(
                pg,
                wgt[nm][:, ch, :],
                combT[:, ch, :],
                start=(ch == 0),
                stop=(ch == NKC - 1),
            )
        sb = cst.tile([HSL, T], f32, name=f"s_{nm}")
        nc.scalar.activation(
            out=sb, in_=pg, func=func, bias=bgc[:, bcol : bcol + 1], scale=1.0
        )
        return sb

    def upd(g, c, nm):
        u = cst.tile([HSL, T], f32, name=f"u_{nm}")
        nc.vector.tensor_mul(u, g, c)
        nc.vector.tensor_sub(u, c, u)
        h = cst.tile([HSL, T], f32, name=f"h_{nm}")
        nc.vector.tensor_tensor_scan(
            out=h, data0=g, data1=u, initial=0.0, op0=OP.mult, op1=OP.add
        )
        return h

    g1 = gate("g1", AF.Sigmoid, 0)
    c1 = gate("c1", AF.Tanh, 1)
    h1 = upd(g1, c1, "1")
    g2 = gate("g2", AF.Sigmoid, 2)
    c2 = gate("c2", AF.Tanh, 3)
    h2 = upd(g2, c2, "2")

    outs = cst.tile([HSL, T], f32)
    nc.vector.tensor_add(outs, h1, h2)

    # weighted time reduction: wsum[h] = sum_t outs[h,t]*w[t]
    scr = cst.tile([HSL, T], f32)
    nc.vector.tensor_mul(scr, outs, wb)
    wsum = cst.tile([HSL, 1], f32)
    nc.vector.tensor_reduce(out=wsum, in_=scr, axis=mybir.AxisListType.X, op=OP.add)

    # partial final projection, d on partitions: out[p, j] = out_d, d=128j+p
    po = pout.tile([128, NJ], f32)
    for j in range(NJ):
        nc.tensor.matmul(
            po[:, j : j + 1],
            wout[:, j * 128 : (j + 1) * 128],
            wsum,
            start=True,
            stop=True,
        )
    ob = cst.tile([128, NJ], f32)
    nc.vector.tensor_copy(ob, po)
    nc.sync.dma_start(out=d_out[:], in_=ob)


def _build_nc_fast():
    """Fast path: pos_emb folded into xT on the host."""
    import concourse.bacc as bacc
    import concourse.tile as tile
    from concourse import mybir

    f32 = mybir.dt.float32
    nc = bacc.Bacc("TRN2", target_bir_lowering=False, debug=False)

    AF = mybir.ActivationFunctionType
    OP = mybir.AluOpType

    f16 = mybir.dt.float16
    d_pA1 = nc.dram_tensor("pack1", [128, _P1_LEN], f32, kind="ExternalInput")
    d_wout = nc.dram_tensor("wout_t", [HSL, OUT_PAD], f16, kind="ExternalInput")
    d_out = nc.dram_tensor("out_part", [128, NJ], f32, kind="ExternalOutput")

    with tile.TileContext(nc) as tc:
        with (
            tc.tile_pool(name="cst", bufs=1) as cst,
            tc.tile_pool(name="pmm", bufs=4, space="PSUM") as pmm,
            tc.tile_pool(name="pout", bufs=1, space="PSUM") as pout,
        ):
            # issue order = transfer order in the DMA pipe: small early pack
            # first, the big late-needed W_out slice last.
            # hoist the ACT function-table load off the critical path: a
            # dummy 1-element sigmoid right at the start makes bacc emit the
            # (1.3us) table load during the input-DMA shadow.
            dummy = cst.tile([1, 1], f32)
            nc.vector.memset(dummy, 0.0)
            nc.scalar.activation(out=dummy, in_=dummy, func=AF.Sigmoid)
            zb = cst.tile([128, 1], f32)
            nc.vector.memset(zb, 0.0)
            # warm up the PE p-state during the same shadow so the real
            # matmuls run at the ramped clock.
            warm = cst.tile([1, 1], f32)
            nc.vector.memset(warm, 0.0)
            pwarm = pmm.tile([1, 1], f32, tag="warm", bufs=1)
            for _ in range(8):
                nc.tensor.matmul(pwarm, warm, warm, start=True, stop=True)

            pA1 = cst.tile([128, _P1_LEN], f32)
            nc.sync.dma_start(out=pA1, in_=d_pA1[:])
            # W_out is stored fp16: halving its bytes pulls the transfer (and
            # its 900ns completion-semaphore latency) fully into the shadow of
            # the gate/scan chain; PSUM still accumulates in f32.
            wout = cst.tile([HSL, OUT_PAD], f16)
            nc.sync.dma_start(out=wout, in_=d_wout[:])

            xTw = pA1[0 : IN_D + 1, _P1_XT:_P1_WGF].bitcast(f16)
            wgf = pA1[0 : IN_D + 1, _P1_WGF:_P1_WB].bitcast(f16)
            wbp = pA1[:, _P1_WB:_P1_LEN].bitcast(f16)

            # paired gates: two matmuls into one PSUM tile, one activation
            # over both halves (biases ride in wgf row 100 against the
            # ones-row of xTw).
            pgg = pmm.tile([HSL, 2 * T], f32, tag="mm", name="pgg")
            nc.tensor.matmul(pgg[:, 0:T], wgf[:, 0 * HSL : 1 * HSL], xTw,
                             start=True, stop=True)
            nc.tensor.matmul(pgg[:, T : 2 * T], wgf[:, 2 * HSL : 3 * HSL], xTw,
                             start=True, stop=True)
            pcc = pmm.tile([HSL, 2 * T], f32, tag="mm", name="pcc")
            nc.tensor.matmul(pcc[:, 0:T], wgf[:, 1 * HSL : 2 * HSL], xTw,
                             start=True, stop=True)
            nc.tensor.matmul(pcc[:, T : 2 * T], wgf[:, 3 * HSL : 4 * HSL], xTw,
                             start=True, stop=True)
            gp = cst.tile([HSL, 2 * T], f16)
            nc.scalar.activation(out=gp, in_=pgg, func=AF.Sigmoid, bias=zb[:, 0:1])
            cp = cst.tile([HSL, 2 * T], f16)
            nc.scalar.activation(out=cp, in_=pcc, func=AF.Tanh, bias=zb[:, 0:1])

            # negu = (g-1)*c in one DVE op; the scan then computes
            # h = g*h - negu = g*h + (1-g)*c directly (op1=subtract).
            up = cst.tile([HSL, 2 * T], f16)
            nc.vector.scalar_tensor_tensor(
                out=up, in0=gp, scalar=1.0, in1=cp,
                op0=OP.subtract, op1=OP.mult,
            )
            hp = cst.tile([HSL, 2 * T], f16)
            nc.vector.tensor_tensor_scan(
                out=hp[:, 0:T], data0=gp[:, 0:T], data1=up[:, 0:T],
                initial=0.0, op0=OP.mult, op1=OP.subtract,
            )
            nc.vector.tensor_tensor_scan(
                out=hp[:, T : 2 * T], data0=gp[:, T : 2 * T], data1=up[:, T : 2 * T],
                initial=0.0, op0=OP.mult, op1=OP.subtract,
            )

            # wsum[h] = sum_t (h1+h2)*w == sum over the whole weighted pair
            scrp = cst.tile([HSL, 2 * T], f16)
            nc.vector.tensor_mul(scrp, hp, wbp)
            wsum16 = cst.tile([HSL, 1], f16)
            with nc.allow_low_precision("128-term f32-accumulated reduce, fp16 store"):
                nc.vector.tensor_reduce(
                    out=wsum16,
                    in_=scrp.rearrange("p (n t) -> p n t", n=2),
                    axis=mybir.AxisListType.XY,
                    op=OP.add,
                )

            po = pout.tile([128, NJ], f32)
            for j in range(NJ):
                nc.tensor.matmul(
                    po[:, j : j + 1],
                    wout[:, j * 128 : (j + 1) * 128],
                    wsum16,
                    start=True,
                    stop=True,
                )
            ob = cst.tile([128, NJ], f32)
            nc.vector.tensor_copy(ob, po)
            nc.sync.dma_start(out=d_out[:], in_=ob)

    nc.finalize()
    return nc


def _build_nc_general():
    """Fallback: full pe stage on device (used when the rank-1 guard fails)."""
    import concourse.bacc as bacc
    import concourse.tile as tile
    from concourse import mybir

    f32 = mybir.dt.float32
    AF = mybir.ActivationFunctionType
    OP = mybir.AluOpType

    nc = bacc.Bacc("TRN2", target_bir_lowering=False, debug=False)

    d_p128 = nc.dram_tensor("pack128", [128, _P128_LEN], f32, kind="ExternalInput")
    d_pe = nc.dram_tensor("pe_pack", [128, 2 * T + 2 * NT], f32, kind="ExternalInput")
    d_w2t = nc.dram_tensor("w2t", [128, NT * IN_D], f32, kind="ExternalInput")
    d_p100 = nc.dram_tensor("pack100", [IN_D, _P100_LEN + 1], f32, kind="ExternalInput")
    d_wout = nc.dram_tensor("wout_t", [HSL, OUT_PAD], f32, kind="ExternalInput")
    d_out = nc.dram_tensor("out_part", [128, NJ], f32, kind="ExternalOutput")

    with tile.TileContext(nc) as tc:
        with (
            tc.tile_pool(name="cst", bufs=1) as cst,
            tc.tile_pool(name="pmm", bufs=2, space="PSUM") as pmm,
            tc.tile_pool(name="pout", bufs=1, space="PSUM") as pout,
        ):
            p128 = cst.tile([128, _P128_LEN], f32)
            nc.sync.dma_start(out=p128, in_=d_p128[:])
            pe_p = cst.tile([128, 2 * T + 2 * NT], f32)
            nc.sync.dma_start(out=pe_p, in_=d_pe[:])
            w2tt = cst.tile([128, NT * IN_D], f32)
            nc.sync.dma_start(out=w2tt, in_=d_w2t[:])
            p100 = cst.tile([IN_D, _P100_LEN + 1], f32)
            nc.sync.dma_start(out=p100, in_=d_p100[:])
            wout = cst.tile([HSL, OUT_PAD], f32)
            nc.scalar.dma_start(out=wout, in_=d_wout[:])

            wb = p128[:, _P128_WB:_P128_BGC]
            bgc = p128[:, _P128_BGC:_P128_LEN]
            posb = pe_p[:, 0:T]
            w1r = pe_p[:, 2 * T : 2 * T + NT]
            b1r = pe_p[:, 2 * T + NT : 2 * T + 2 * NT]
            w2t = w2tt.rearrange("p (n k) -> p n k", n=NT)

            tsT = p100[:, _P100_XT:_P100_WP1]
            b2c = p100[:, _P100_LEN : _P100_LEN + 1]
            wp1 = p100[:, _P100_WP1:_P100_BP1]
            bp1 = p100[:, _P100_BP1:_P100_WG]
            wgt = {}
            for gi, nm in enumerate(("g1", "c1", "g2", "c2")):
                o = _P100_WG + gi * NKC * HSL
                wgt[nm] = p100[:, o : o + NKC * HSL].rearrange(
                    "p (n m) -> p n m", n=NKC
                )

            # pe stage: peT[h, t] = relu(pos_t*w1[h]+b1[h]); pos_embT = sum_h
            peT = cst.tile([128, NT, T], f32)
            for i in range(NT):
                nc.scalar.activation(
                    out=peT[:, i, :],
                    in_=posb,
                    func=AF.Relu,
                    bias=b1r[:, i : i + 1],
                    scale=w1r[:, i : i + 1],
                )
            ppe = pmm.tile([IN_D, T], f32, tag="mm")
            for i in range(NT):
                nc.tensor.matmul(
                    ppe, w2t[:, i, :], peT[:, i, :], start=(i == 0), stop=(i == NT - 1)
                )
            xT = cst.tile([IN_D, T], f32)
            nc.vector.scalar_tensor_tensor(
                out=xT, in0=ppe, scalar=b2c[:, 0:1], in1=tsT, op0=OP.add, op1=OP.add
            )

            _build_core(
                nc, tile, mybir, tc, cst, pmm, pout,
                xT, wp1, bp1, wgt, bgc, wb, wout, d_out,
            )

    nc.finalize()
    return nc


def _prep_common(inputs):
    f = np.float32
    arr = {k: np.asarray(v, dtype=f) for k, v in inputs.items() if k != "positions"}
    pos = np.asarray(inputs["positions"]).astype(f)
    ts = arr["time_steps"]
    S = ts.shape[0]
    # softmax(arange(S,0,-1))[t] = exp(-t)/Z with Z the geometric sum.
    Z = (1.0 - np.exp(-float(S))) / (1.0 - np.exp(-1.0))
    w = (np.exp(-np.arange(T, dtype=np.float64)) / Z).astype(f)
    return arr, pos, w


def _core_p128(a, p128_base, sl):
    pc = p128_base.copy()
    pc[:, _P128_BGC + 0] = a["b_g1"][sl]
    pc[:, _P128_BGC + 1] = a["b_c1"][sl]
    pc[:, _P128_BGC + 2] = a["b_g2"][sl]
    pc[:, _P128_BGC + 3] = a["b_c2"][sl]
    return pc


def _core_wg(a, sl):
    wg = np.zeros((IN_D, _WG_LEN), np.float32)
    for gi, k in enumerate(("W_g1", "W_c1", "W_g2", "W_c2")):
        o = gi * NKC * HSL
        blk = a[k][sl].T.reshape(NKC, 100, HSL).transpose(1, 0, 2)
        wg[:, o : o + NKC * HSL] = blk.reshape(100, NKC * HSL)
    return wg


def _core_wout(a, sl, dtype=np.float16):
    wo = np.zeros((HSL, OUT_PAD), dtype)
    wo[:, :OUT_D] = a["W_out"][:, sl].T.astype(dtype)
    return wo


def _prep_inputs(inputs):
    """Host-side shard/layout prep. Returns (mode, per-core input maps, b_out)."""
    a, pos, w = _prep_common(inputs)
    ts = a["time_steps"]

    p128 = np.zeros((128, _P128_LEN), np.float32)
    p128[:, _P128_WB:_P128_BGC] = w[None, :]

    fast = bool((a["b_pe1"] == 0).all() and (pos[:T] >= 0).all())
    if fast:
        # rank-1 pos_emb folded into xT (see module docstring)
        v = a["W_pe2"] @ np.maximum(a["W_pe1"][:, 0], 0.0)
        xT = ts[:T].T + v[:, None] * pos[None, :T] + a["b_pe2"][:, None]
        # fold proj1 into the gate weights/biases (linear-layer composition)
        Wf = {k: a[k] @ a["W_p1"] for k in ("W_g1", "W_c1", "W_g2", "W_c2")}
        bf = {
            "b_g1": a["b_g1"] + a["W_g1"] @ a["b_p1"],
            "b_c1": a["b_c1"] + a["W_c1"] @ a["b_p1"],
            "b_g2": a["b_g2"] + a["W_g2"] @ a["b_p1"],
            "b_c2": a["b_c2"] + a["W_c2"] @ a["b_p1"],
        }
        af = dict(a)
        af.update(bf)
        in_maps = []
        for ci in range(NCORES):
            sl = slice(ci * HSL, (ci + 1) * HSL)
            pa1 = np.zeros((128, _P1_LEN), np.float32)
            h16 = pa1.view(np.float16)
            h16[:IN_D, 0:T] = xT.astype(np.float16)
            h16[IN_D, 0:T] = 1.0
            for gi, k in enumerate(("W_g1", "W_c1", "W_g2", "W_c2")):
                o = 2 * _P1_WGF + gi * HSL
                h16[:IN_D, o : o + HSL] = Wf[k][sl].T.astype(np.float16)
                h16[IN_D, o : o + HSL] = bf["b" + k[1:]][sl].astype(np.float16)
            w16 = w.astype(np.float16)
            h16[:, 2 * _P1_WB : 2 * _P1_WB + T] = w16[None, :]
            h16[:, 2 * _P1_WB + T : 2 * _P1_WB + 2 * T] = w16[None, :]
            in_maps.append({
                "pack1": pa1,
                "wout_t": _core_wout(a, sl),
            })
        return "fast", in_maps, a["b_out"]

    # general fallback: pe stage on device
    pe_p = np.zeros((128, 2 * T + 2 * NT), np.float32)
    pe_p[:, 0:T] = pos[None, :T]
    pe_p[:, 2 * T : 2 * T + NT] = a["W_pe1"][:, 0].reshape(NT, 128).T
    pe_p[:, 2 * T + NT : 2 * T + 2 * NT] = a["b_pe1"].reshape(NT, 128).T
    w2t = (
        a["W_pe2"].T.reshape(NT, 128, IN_D).transpose(1, 0, 2).reshape(128, NT * IN_D)
    ).copy()
    p100 = np.zeros((IN_D, _P100_LEN + 1), np.float32)
    p100[:, _P100_XT:_P100_WP1] = ts[:T].T
    p100[:, _P100_WP1:_P100_BP1] = a["W_p1"].T
    p100[:, _P100_BP1:_P100_WG] = a["b_p1"].reshape(NKC, 100).T
    p100[:, _P100_LEN] = a["b_pe2"]
    in_maps = []
    for ci in range(NCORES):
        sl = slice(ci * HSL, (ci + 1) * HSL)
        full = p100.copy()
        full[:, _P100_WG:_P100_LEN] = _core_wg(a, sl)
        in_maps.append({
            "pack128": _core_p128(a, p128, sl),
            "pack100": full,
            "pe_pack": pe_p,
            "w2t": w2t,
            "wout_t": _core_wout(a, sl, dtype=np.float32),
        })
    return "general", in_maps, a["b_out"]


def _run(inputs, trace=False):
    from concourse.bass_utils import run_bass_kernel_spmd

    mode, in_maps, b_out = _prep_inputs(inputs)
    key = f"nc_{mode}"
    if key not in _CACHE:
        _CACHE[key] = _build_nc_fast() if mode == "fast" else _build_nc_general()
    nc = _CACHE[key]
    res = run_bass_kernel_spmd(nc, in_maps, core_ids=list(range(NCORES)), trace=trace)
    acc = np.zeros(OUT_D, dtype=np.float32)
    for r in res.results:
        acc = acc + r["out_part"].T.ravel()[:OUT_D]
    return (acc + b_out).astype(np.float32), res


def kernel(**inputs):
    out, _ = _run(inputs, trace=False)
    return out


# revision 27
# speedup vs baseline: 1.0253x; 1.0253x over previous
"""Trainium2 Bass kernel for nn_EnhancedGatedTemporalFusion.

Mathematical structure exploited (all exact at f32 precision):
  * The self-attention block in the reference is dead code (its result is
    never used downstream), so it is skipped.
  * The output weighting is softmax(arange(S,0,-1)), i.e. w[t] = exp(-t)/Z.
    Since |outputs[t]| <= 2 (convex combinations of tanh values), the tail
    sum over t >= T is bounded by 2*e^{-T}; at T=32 that is ~2.5e-14
    absolute against a result of magnitude ~0.05 (f32 ulp ~4e-9), below one
    ulp; the fp16 storage of w[t] zeroes t>=17 anyway (tail ~2e-6 relative,
    far below the fp16 weight-quantization noise).
  * The gated update h' = g*h + (1-g)*c is an affine recurrence that maps
    1:1 onto the DVE TensorTensorScanArith instruction
    (state = (g mult state) add u, scanned along the free dimension).
  * When b_pe1 == 0 and positions >= 0 (true for this problem's inputs),
    relu(pos_t*w1[h]) = pos_t*relu(w1[h]), so the positional-encoding MLP
    is rank-1: pos_emb[t, :] = pos_t * (W_pe2 @ relu(W_pe1[:, 0])).  That
    O(T*IN_D) correction is folded into the x input on the host, removing
    the whole 1024-wide pe stage from the device graph.  A general device
    path is kept as a fallback and used automatically if the guard fails.

Sharding across the 8 cores: the hidden dim H=1024 is split 128 lanes per
core.  Each core computes the (replicated, tiny) 300-wide projection, its
own h-slice of the gate/candidate GEMMs, the scan, the exp-weighted time
reduction, and a partial product of the final H->2613 projection over its
h-slice.  The 8 partial vectors are summed on the host (contraction
unshard) and the output bias is added.

Layout notes: device inputs are pre-packed on the host into a few DMA
transfers ordered by when the consuming compute needs them (each dma_start
costs ~625ns of HWDGE issue time, so many small DMAs serialize the front).
The final projection keeps the output dim on PSUM partitions ((128, 21)
tile, d = 128*j + p) so the PSUM->SBUF copy is partition-parallel; the
host undoes that tiling when gathering.
"""

import sys

import numpy as np

if "/opt/trn_rl_repo" not in sys.path:
    sys.path.insert(0, "/opt/trn_rl_repo")

T = 32           # truncated horizon (exact under f32, see module docstring)
IN_D = 100       # input_dim
CH = 300         # proj1 out dim
H = 1024         # hidden dim
OUT_D = 2613     # output dim
OUT_PAD = 2688   # 21 * 128
NJ = OUT_PAD // 128
NCORES = 8
HSL = H // NCORES      # h-lanes per core
NT = H // 128          # h tiles of 128 for the fallback pe stage
NKC = CH // 100        # contraction chunks of 100 for the 300-dim

# pack128 free-dim offsets (wb + per-core gate biases)
_P128_WB = 0
_P128_BGC = _P128_WB + T
_P128_LEN = _P128_BGC + 4

# fast path: packA carries wb + gate biases + xT + fused gate weights.
# wb, xT and the gate weights are fp16 bit-packed into the f32 pack (halves
# the front DMA, runs gate matmuls at 1 cycle/row, 2x DVE modes downstream).
# pack1: fp16 [xT | ones-row] (101, T) + fp16 augmented gate weights
# (101, 4*HSL) with the fused biases in row 100 (K=101 matmul adds them).
_P1_XT = 0
_P1_WGF = _P1_XT + T // 2
_P1_WB = _P1_WGF + 4 * HSL // 2       # fp16 softmax weights, duplicated (128, 2T)
_P1_LEN = _P1_WB + T

# general-path offsets
_P100_XT = 0
_P100_WP1 = _P100_XT + T
_P100_BP1 = _P100_WP1 + CH
_P100A_LEN = _P100_BP1 + NKC
_WG_LEN = 4 * NKC * HSL
_P100_WG = _P100A_LEN
_P100_LEN = _P100_WG + _WG_LEN

_CACHE = {}


def _build_core(nc, tile, mybir, tc, cst, pmm, pout, xT, wp1, bp1, wgt, bgc, wb, wout, d_out):
    """Shared back end: combined -> gates -> scan -> weighted sum -> partial out."""
    f32 = mybir.dt.float32
    AF = mybir.ActivationFunctionType
    OP = mybir.AluOpType

    # combinedT (ch=300 in 3 chunks of 100, t)
    combT = cst.tile([100, NKC, T], f32)
    for ch in range(NKC):
        pcomb = pmm.tile([100, T], f32, tag="mm", name=f"pcomb{ch}")
        nc.tensor.matmul(
            pcomb, wp1[:, ch * 100 : (ch + 1) * 100], xT, start=True, stop=True
        )
        nc.vector.tensor_scalar_add(combT[:, ch, :], pcomb, bp1[:, ch : ch + 1])

    def gate(nm, func, bcol):
        pg = pmm.tile([HSL, T], f32, tag="mm", name=f"p_{nm}")
        for ch in range(NKC):
            nc.tensor.matmul(
                pg,
                wgt[nm][:, ch, :],
                combT[:, ch, :],
                start=(ch == 0),
                stop=(ch == NKC - 1),
            )
        sb = cst.tile([HSL, T], f32, name=f"s_{nm}")
        nc.scalar.activation(
            out=sb, in_=pg, func=func, bias=bgc[:, bcol : bcol + 1], scale=1.0
        )
        return sb

    def upd(g, c, nm):
        u = cst.tile([HSL, T], f32, name=f"u_{nm}")
        nc.vector.tensor_mul(u, g, c)
        nc.vector.tensor_sub(u, c, u)
        h = cst.tile([HSL, T], f32, name=f"h_{nm}")
        nc.vector.tensor_tensor_scan(
            out=h, data0=g, data1=u, initial=0.0, op0=OP.mult, op1=OP.add
        )
        return h

    g1 = gate("g1", AF.Sigmoid, 0)
    c1 = gate("c1", AF.Tanh, 1)
    h1 = upd(g1, c1, "1")
    g2 = gate("g2", AF.Sigmoid, 2)
    c2 = gate("c2", AF.Tanh, 3)
    h2 = upd(g2, c2, "2")

    outs = cst.tile([HSL, T], f32)
    nc.vector.tensor_add(outs, h1, h2)

    # weighted time reduction: wsum[h] = sum_t outs[h,t]*w[t]
    scr = cst.tile([HSL, T], f32)
    nc.vector.tensor_mul(scr, outs, wb)
    wsum = cst.tile([HSL, 1], f32)
    nc.vector.tensor_reduce(out=wsum, in_=scr, axis=mybir.AxisListType.X, op=OP.add)

    # partial final projection, d on partitions: out[p, j] = out_d, d=128j+p
    po = pout.tile([128, NJ], f32)
    for j in range(NJ):
        nc.tensor.matmul(
            po[:, j : j + 1],
            wout[:, j * 128 : (j + 1) * 128],
            wsum,
            start=True,
            stop=True,
        )
    ob = cst.tile([128, NJ], f32)
    nc.vector.tensor_copy(ob, po)
    nc.sync.dma_start(out=d_out[:], in_=ob)


def _build_nc_fast():
    """Fast path: pos_emb folded into xT on the host."""
    import concourse.bacc as bacc
    import concourse.tile as tile
    from concourse import mybir

    f32 = mybir.dt.float32
    nc = bacc.Bacc("TRN2", target_bir_lowering=False, debug=False)

    AF = mybir.ActivationFunctionType
    OP = mybir.AluOpType

    f16 = mybir.dt.float16
    d_pA1 = nc.dram_tensor("pack1", [128, _P1_LEN], f32, kind="ExternalInput")
    d_wout = nc.dram_tensor("wout_t", [HSL, OUT_PAD], f16, kind="ExternalInput")
    d_out = nc.dram_tensor("out_part", [128, NJ], f32, kind="ExternalOutput")

    with tile.TileContext(nc) as tc:
        with (
            tc.tile_pool(name="cst", bufs=1) as cst,
            tc.tile_pool(name="pmm", bufs=4, space="PSUM") as pmm,
            tc.tile_pool(name="pout", bufs=1, space="PSUM") as pout,
        ):
            # issue order = transfer order in the DMA pipe: small early pack
            # first, the big late-needed W_out slice last.
            # hoist the ACT function-table load off the critical path: a
            # dummy 1-element sigmoid right at the start makes bacc emit the
            # (1.3us) table load during the input-DMA shadow.
            dummy = cst.tile([1, 1], f32)
            nc.vector.memset(dummy, 0.0)
            nc.scalar.activation(out=dummy, in_=dummy, func=AF.Sigmoid)
            zb = cst.tile([128, 1], f32)
            nc.vector.memset(zb, 0.0)
            # warm up the PE p-state during the same shadow so the real
            # matmuls run at the ramped clock.
            warm = cst.tile([1, 1], f32)
            nc.vector.memset(warm, 0.0)
            pwarm = pmm.tile([1, 1], f32, tag="warm", bufs=1)
            for _ in range(8):
                nc.tensor.matmul(pwarm, warm, warm, start=True, stop=True)

            pA1 = cst.tile([128, _P1_LEN], f32)
            nc.sync.dma_start(out=pA1, in_=d_pA1[:])
            # W_out is stored fp16: halving its bytes pulls the transfer (and
            # its 900ns completion-semaphore latency) mostly into the shadow
            # of the gate/scan chain; PSUM still accumulates in f32.  Issued
            # via SWDGE (Pool) so its descriptor generation runs in parallel
            # with pack1's HWDGE issue instead of queueing behind it.
            wout = cst.tile([HSL, OUT_PAD], f16)
            nc.gpsimd.dma_start(out=wout, in_=d_wout[:])

            xTw = pA1[0 : IN_D + 1, _P1_XT:_P1_WGF].bitcast(f16)
            wgf = pA1[0 : IN_D + 1, _P1_WGF:_P1_WB].bitcast(f16)
            wbp = pA1[:, _P1_WB:_P1_LEN].bitcast(f16)

            # paired gates: two matmuls into one PSUM tile, one activation
            # over both halves (biases ride in wgf row 100 against the
            # ones-row of xTw).
            pgg = pmm.tile([HSL, 2 * T], f32, tag="mm", name="pgg")
            nc.tensor.matmul(pgg[:, 0:T], wgf[:, 0 * HSL : 1 * HSL], xTw,
                             start=True, stop=True)
            nc.tensor.matmul(pgg[:, T : 2 * T], wgf[:, 2 * HSL : 3 * HSL], xTw,
                             start=True, stop=True)
            pcc = pmm.tile([HSL, 2 * T], f32, tag="mm", name="pcc")
            nc.tensor.matmul(pcc[:, 0:T], wgf[:, 1 * HSL : 2 * HSL], xTw,
                             start=True, stop=True)
            nc.tensor.matmul(pcc[:, T : 2 * T], wgf[:, 3 * HSL : 4 * HSL], xTw,
                             start=True, stop=True)
            gp = cst.tile([HSL, 2 * T], f16)
            nc.scalar.activation(out=gp, in_=pgg, func=AF.Sigmoid, bias=zb[:, 0:1])
            cp = cst.tile([HSL, 2 * T], f16)
            nc.scalar.activation(out=cp, in_=pcc, func=AF.Tanh, bias=zb[:, 0:1])

            # negu = (g-1)*c in one DVE op; the scan then computes
            # h = g*h - negu = g*h + (1-g)*c directly (op1=subtract).
            up = cst.tile([HSL, 2 * T], f16)
            nc.vector.scalar_tensor_tensor(
                out=up, in0=gp, scalar=1.0, in1=cp,
                op0=OP.subtract, op1=OP.mult,
            )
            hp = cst.tile([HSL, 2 * T], f16)
            nc.vector.tensor_tensor_scan(
                out=hp[:, 0:T], data0=gp[:, 0:T], data1=up[:, 0:T],
                initial=0.0, op0=OP.mult, op1=OP.subtract,
            )
            nc.vector.tensor_tensor_scan(
                out=hp[:, T : 2 * T], data0=gp[:, T : 2 * T], data1=up[:, T : 2 * T],
                initial=0.0, op0=OP.mult, op1=OP.subtract,
            )

            # wsum[h] = sum_t (h1+h2)*w == sum over the whole weighted pair
            scrp = cst.tile([HSL, 2 * T], f16)
            nc.vector.tensor_mul(scrp, hp, wbp)
            wsum16 = cst.tile([HSL, 1], f16)
            with nc.allow_low_precision("128-term f32-accumulated reduce, fp16 store"):
                nc.vector.tensor_reduce(
                    out=wsum16,
                    in_=scrp.rearrange("p (n t) -> p n t", n=2),
                    axis=mybir.AxisListType.XY,
                    op=OP.add,
                )

            po = pout.tile([128, NJ], f32)
            for j in range(NJ):
                nc.tensor.matmul(
                    po[:, j : j + 1],
                    wout[:, j * 128 : (j + 1) * 128],
                    wsum16,
                    start=True,
                    stop=True,
                )
            ob = cst.tile([128, NJ], f32)
            nc.vector.tensor_copy(ob, po)
            nc.sync.dma_start(out=d_out[:], in_=ob)

    nc.finalize()
    return nc


def _build_nc_general():
    """Fallback: full pe stage on device (used when the rank-1 guard fails)."""
    import concourse.bacc as bacc
    import concourse.tile as tile
    from concourse import mybir

    f32 = mybir.dt.float32
    AF = mybir.ActivationFunctionType
    OP = mybir.AluOpType

    nc = bacc.Bacc("TRN2", target_bir_lowering=False, debug=False)

    d_p128 = nc.dram_tensor("pack128", [128, _P128_LEN], f32, kind="ExternalInput")
    d_pe = nc.dram_tensor("pe_pack", [128, 2 * T + 2 * NT], f32, kind="ExternalInput")
    d_w2t = nc.dram_tensor("w2t", [128, NT * IN_D], f32, kind="ExternalInput")
    d_p100 = nc.dram_tensor("pack100", [IN_D, _P100_LEN + 1], f32, kind="ExternalInput")
    d_wout = nc.dram_tensor("wout_t", [HSL, OUT_PAD], f32, kind="ExternalInput")
    d_out = nc.dram_tensor("out_part", [128, NJ], f32, kind="ExternalOutput")

    with tile.TileContext(nc) as tc:
        with (
            tc.tile_pool(name="cst", bufs=1) as cst,
            tc.tile_pool(name="pmm", bufs=2, space="PSUM") as pmm,
            tc.tile_pool(name="pout", bufs=1, space="PSUM") as pout,
        ):
            p128 = cst.tile([128, _P128_LEN], f32)
            nc.sync.dma_start(out=p128, in_=d_p128[:])
            pe_p = cst.tile([128, 2 * T + 2 * NT], f32)
            nc.sync.dma_start(out=pe_p, in_=d_pe[:])
            w2tt = cst.tile([128, NT * IN_D], f32)
            nc.sync.dma_start(out=w2tt, in_=d_w2t[:])
            p100 = cst.tile([IN_D, _P100_LEN + 1], f32)
            nc.sync.dma_start(out=p100, in_=d_p100[:])
            wout = cst.tile([HSL, OUT_PAD], f32)
            nc.scalar.dma_start(out=wout, in_=d_wout[:])

            wb = p128[:, _P128_WB:_P128_BGC]
            bgc = p128[:, _P128_BGC:_P128_LEN]
            posb = pe_p[:, 0:T]
            w1r = pe_p[:, 2 * T : 2 * T + NT]
            b1r = pe_p[:, 2 * T + NT : 2 * T + 2 * NT]
            w2t = w2tt.rearrange("p (n k) -> p n k", n=NT)

            tsT = p100[:, _P100_XT:_P100_WP1]
            b2c = p100[:, _P100_LEN : _P100_LEN + 1]
            wp1 = p100[:, _P100_WP1:_P100_BP1]
            bp1 = p100[:, _P100_BP1:_P100_WG]
            wgt = {}
            for gi, nm in enumerate(("g1", "c1", "g2", "c2")):
                o = _P100_WG + gi * NKC * HSL
                wgt[nm] = p100[:, o : o + NKC * HSL].rearrange(
                    "p (n m) -> p n m", n=NKC
                )

            # pe stage: peT[h, t] = relu(pos_t*w1[h]+b1[h]); pos_embT = sum_h
            peT = cst.tile([128, NT, T], f32)
            for i in range(NT):
                nc.scalar.activation(
                    out=peT[:, i, :],
                    in_=posb,
                    func=AF.Relu,
                    bias=b1r[:, i : i + 1],
                    scale=w1r[:, i : i + 1],
                )
            ppe = pmm.tile([IN_D, T], f32, tag="mm")
            for i in range(NT):
                nc.tensor.matmul(
                    ppe, w2t[:, i, :], peT[:, i, :], start=(i == 0), stop=(i == NT - 1)
                )
            xT = cst.tile([IN_D, T], f32)
            nc.vector.scalar_tensor_tensor(
                out=xT, in0=ppe, scalar=b2c[:, 0:1], in1=tsT, op0=OP.add, op1=OP.add
            )

            _build_core(
                nc, tile, mybir, tc, cst, pmm, pout,
                xT, wp1, bp1, wgt, bgc, wb, wout, d_out,
            )

    nc.finalize()
    return nc


def _prep_common(inputs):
    f = np.float32
    arr = {k: np.asarray(v, dtype=f) for k, v in inputs.items() if k != "positions"}
    pos = np.asarray(inputs["positions"]).astype(f)
    ts = arr["time_steps"]
    S = ts.shape[0]
    # softmax(arange(S,0,-1))[t] = exp(-t)/Z with Z the geometric sum.
    Z = (1.0 - np.exp(-float(S))) / (1.0 - np.exp(-1.0))
    w = (np.exp(-np.arange(T, dtype=np.float64)) / Z).astype(f)
    return arr, pos, w


def _core_p128(a, p128_base, sl):
    pc = p128_base.copy()
    pc[:, _P128_BGC + 0] = a["b_g1"][sl]
    pc[:, _P128_BGC + 1] = a["b_c1"][sl]
    pc[:, _P128_BGC + 2] = a["b_g2"][sl]
    pc[:, _P128_BGC + 3] = a["b_c2"][sl]
    return pc


def _core_wg(a, sl):
    wg = np.zeros((IN_D, _WG_LEN), np.float32)
    for gi, k in enumerate(("W_g1", "W_c1", "W_g2", "W_c2")):
        o = gi * NKC * HSL
        blk = a[k][sl].T.reshape(NKC, 100, HSL).transpose(1, 0, 2)
        wg[:, o : o + NKC * HSL] = blk.reshape(100, NKC * HSL)
    return wg


def _core_wout(a, sl, dtype=np.float16):
    wo = np.zeros((HSL, OUT_PAD), dtype)
    wo[:, :OUT_D] = a["W_out"][:, sl].T.astype(dtype)
    return wo


def _prep_inputs(inputs):
    """Host-side shard/layout prep. Returns (mode, per-core input maps, b_out)."""
    a, pos, w = _prep_common(inputs)
    ts = a["time_steps"]

    p128 = np.zeros((128, _P128_LEN), np.float32)
    p128[:, _P128_WB:_P128_BGC] = w[None, :]

    fast = bool((a["b_pe1"] == 0).all() and (pos[:T] >= 0).all())
    if fast:
        # rank-1 pos_emb folded into xT (see module docstring)
        v = a["W_pe2"] @ np.maximum(a["W_pe1"][:, 0], 0.0)
        xT = ts[:T].T + v[:, None] * pos[None, :T] + a["b_pe2"][:, None]
        # fold proj1 into the gate weights/biases (linear-layer composition)
        Wf = {k: a[k] @ a["W_p1"] for k in ("W_g1", "W_c1", "W_g2", "W_c2")}
        bf = {
            "b_g1": a["b_g1"] + a["W_g1"] @ a["b_p1"],
            "b_c1": a["b_c1"] + a["W_c1"] @ a["b_p1"],
            "b_g2": a["b_g2"] + a["W_g2"] @ a["b_p1"],
            "b_c2": a["b_c2"] + a["W_c2"] @ a["b_p1"],
        }
        af = dict(a)
        af.update(bf)
        in_maps = []
        for ci in range(NCORES):
            sl = slice(ci * HSL, (ci + 1) * HSL)
            pa1 = np.zeros((128, _P1_LEN), np.float32)
            h16 = pa1.view(np.float16)
            h16[:IN_D, 0:T] = xT.astype(np.float16)
            h16[IN_D, 0:T] = 1.0
            for gi, k in enumerate(("W_g1", "W_c1", "W_g2", "W_c2")):
                o = 2 * _P1_WGF + gi * HSL
                h16[:IN_D, o : o + HSL] = Wf[k][sl].T.astype(np.float16)
                h16[IN_D, o : o + HSL] = bf["b" + k[1:]][sl].astype(np.float16)
            w16 = w.astype(np.float16)
            h16[:, 2 * _P1_WB : 2 * _P1_WB + T] = w16[None, :]
            h16[:, 2 * _P1_WB + T : 2 * _P1_WB + 2 * T] = w16[None, :]
            in_maps.append({
                "pack1": pa1,
                "wout_t": _core_wout(a, sl),
            })
        return "fast", in_maps, a["b_out"]

    # general fallback: pe stage on device
    pe_p = np.zeros((128, 2 * T + 2 * NT), np.float32)
    pe_p[:, 0:T] = pos[None, :T]
    pe_p[:, 2 * T : 2 * T + NT] = a["W_pe1"][:, 0].reshape(NT, 128).T
    pe_p[:, 2 * T + NT : 2 * T + 2 * NT] = a["b_pe1"].reshape(NT, 128).T
    w2t = (
        a["W_pe2"].T.reshape(NT, 128, IN_D).transpose(1, 0, 2).reshape(128, NT * IN_D)
    ).copy()
    p100 = np.zeros((IN_D, _P100_LEN + 1), np.float32)
    p100[:, _P100_XT:_P100_WP1] = ts[:T].T
    p100[:, _P100_WP1:_P100_BP1] = a["W_p1"].T
    p100[:, _P100_BP1:_P100_WG] = a["b_p1"].reshape(NKC, 100).T
    p100[:, _P100_LEN] = a["b_pe2"]
    in_maps = []
    for ci in range(NCORES):
        sl = slice(ci * HSL, (ci + 1) * HSL)
        full = p100.copy()
        full[:, _P100_WG:_P100_LEN] = _core_wg(a, sl)
        in_maps.append({
            "pack128": _core_p128(a, p128, sl),
            "pack100": full,
            "pe_pack": pe_p,
            "w2t": w2t,
            "wout_t": _core_wout(a, sl, dtype=np.float32),
        })
    return "general", in_maps, a["b_out"]


def _run(inputs, trace=False):
    from concourse.bass_utils import run_bass_kernel_spmd

    mode, in_maps, b_out = _prep_inputs(inputs)
    key = f"nc_{mode}"
    if key not in _CACHE:
        _CACHE[key] = _build_nc_fast() if mode == "fast" else _build_nc_general()
    nc = _CACHE[key]
    res = run_bass_kernel_spmd(nc, in_maps, core_ids=list(range(NCORES)), trace=trace)
    acc = np.zeros(OUT_D, dtype=np.float32)
    for r in res.results:
        acc = acc + r["out_part"].T.ravel()[:OUT_D]
    return (acc + b_out).astype(np.float32), res


def kernel(**inputs):
    out, _ = _run(inputs, trace=False)
    return out
